# revision 1
# baseline (speedup 1.0000x reference)
"""Trainium2 Bass kernel for MFVIConstituency mean-field iterations.

Per batch b (one NeuronCore each, 8 total):
    q = s_con;  repeat 3x:  q[i,j] = s_con[i,j] + sum_k sig(q)[j,k] * sb[i,j,k]
    out = sigmoid(q)
where sb = s_bin * mask2o, mask2o[i,j,k] = mask[i,j] & (i!=k) & (j!=k).

Host (numpy) does: masking, fp16 cast, SBUF-cache layout packing, iteration-1
sigmoid, final transpose. Device does, per iteration: fp16 tensor_tensor mul
(DVE 2x mode) -> segmented reduction split between a DVE in-place pairwise
tree (fp16 adds at 2x) and ACT activation-accumulate, then sigmoid + xbar
transposes to rebuild the sig operand layout.

On-chip layout: q is assembled transposed (QT[j,i]); j lives on partitions in
two chunks: chunk1 = j 0:128, chunk2 "packed" = j 128:192 duplicated across
both partition halves with the i-range split (p<64: i 0:96, p>=64: i 96:192)
so every DVE instruction uses all 128 partitions.
"""

import numpy as np

S = 192
B = 8
P = 128
G = 48          # i-values per slab -> slab free size G*S = 9216
NSLAB1 = 4      # chunk1: 4 slabs of 48 i-values (j in 0:128)
NSLAB2 = 2      # chunk2 packed: 96 i-per-half * 2 halves / 48
DVE_SEGS = 34   # per slab: segments reduced by the DVE tree; rest go to ACT
SLAB_ORDER = [4, 5, 0, 1, 2, 3]   # chunk2 first so its boundary work overlaps

_CACHE = {}


def _build_program():
    import concourse.tile as tile
    from concourse import mybir, bacc
    from contextlib import ExitStack

    f32, f16 = mybir.dt.float32, mybir.dt.float16
    SLAB = G * S
    Sig = None

    nc = bacc.Bacc("TRN2", target_bir_lowering=False, debug=False, num_devices=B)
    Sig = __import__("concourse.mybir", fromlist=["x"]).ActivationFunctionType.Sigmoid
    Cpy = __import__("concourse.mybir", fromlist=["x"]).ActivationFunctionType.Copy
    c1_d = nc.dram_tensor("c1", [P, NSLAB1 * SLAB], f16, kind="ExternalInput")
    c2_d = nc.dram_tensor("c2", [P, NSLAB2 * SLAB], f16, kind="ExternalInput")
    siga_d = nc.dram_tensor("siga", [P, S], f16, kind="ExternalInput")
    sigb_d = nc.dram_tensor("sigb", [P, S], f16, kind="ExternalInput")
    sconT1_d = nc.dram_tensor("sconT1", [P, S], f32, kind="ExternalInput")
    sconT2p_d = nc.dram_tensor("sconT2p", [P, 96], f32, kind="ExternalInput")
    qt_d = nc.dram_tensor("qt_out", [S, S], f32, kind="ExternalOutput")

    with tile.TileContext(nc) as tc, ExitStack() as ctx:
        cache_p = ctx.enter_context(tc.tile_pool(name="cache", bufs=1))
        small_p = ctx.enter_context(tc.tile_pool(name="small", bufs=1))
        sig_p = ctx.enter_context(tc.tile_pool(name="sig", bufs=2))
        qt_p = ctx.enter_context(tc.tile_pool(name="qt", bufs=2))
        p_p = ctx.enter_context(tc.tile_pool(name="prod", bufs=4))
        junk_p = ctx.enter_context(tc.tile_pool(name="junk", bufs=4))
        sq_p = ctx.enter_context(tc.tile_pool(name="sq", bufs=2))
        out_p = ctx.enter_context(tc.tile_pool(name="out", bufs=1))

        sconT1_t = small_p.tile([P, S], f32, tag="sc1")
        nc.scalar.dma_start(sconT1_t[:], sconT1_d.ap())
        sconT2p_t = small_p.tile([P, 96], f32, tag="sc2")
        nc.scalar.dma_start(sconT2p_t[:], sconT2p_d.ap())
        siga_t = sig_p.tile([P, S], f16, tag="siga")
        nc.scalar.dma_start(siga_t[:], siga_d.ap())
        sigb_t = sig_p.tile([P, S], f16, tag="sigb")
        nc.scalar.dma_start(sigb_t[:], sigb_d.ap())

        cts = {}
        for idx, s in enumerate(SLAB_ORDER):
            ct = cache_p.tile([P, SLAB], f16, tag=f"c{s}")
            if s < NSLAB1:
                src = c1_d.ap()[:, s * SLAB:(s + 1) * SLAB]
            else:
                src = c2_d.ap()[:, (s - NSLAB1) * SLAB:(s - NSLAB1 + 1) * SLAB]
            eng = nc.sync
            if idx < 2:
                # split first-wave loads so compute ramps sooner
                h = SLAB // 2
                eng.dma_start(ct[:, 0:h], src[:, 0:h])
                eng.dma_start(ct[:, h:SLAB], src[:, h:SLAB])
            else:
                eng.dma_start(ct[:], src)
            cts[s] = ct

        def do_slab(s, siga_t, sigb_t, qt1, qt2, split=None):
            is1 = s < NSLAB1
            sig_t = siga_t if is1 else sigb_t
            qt_t = qt1 if is1 else qt2
            base = (s if is1 else s - NSLAB1) * G
            pt = p_p.tile([P, SLAB], f16)
            p3 = pt[:].rearrange("p (g k) -> p g k", k=S)
            in0 = cts[s][:].rearrange("p (g k) -> p g k", k=S)
            in1 = sig_t[:].unsqueeze(1).broadcast_to([P, G, S])
            if split == "g":       # ramp: match the halved first-wave DMAs
                h = G // 2
                nc.vector.tensor_tensor(p3[:, 0:h, :], in0[:, 0:h, :],
                                        in1[:, 0:h, :], mybir.AluOpType.mult)
                nc.vector.tensor_tensor(p3[:, h:G, :], in0[:, h:G, :],
                                        in1[:, h:G, :], mybir.AluOpType.mult)
            elif split == "k":     # boundary: high k-columns are ready first
                nc.vector.tensor_tensor(p3[:, :, 128:S], in0[:, :, 128:S],
                                        in1[:, :, 128:S], mybir.AluOpType.mult)
                nc.vector.tensor_tensor(p3[:, :, 0:128], in0[:, :, 0:128],
                                        in1[:, :, 0:128], mybir.AluOpType.mult)
            else:
                nc.vector.tensor_tensor(p3, in0, in1, mybir.AluOpType.mult)
            d = DVE_SEGS
            if d > 0:
                w = S
                while w > 3:   # in-place fp16 pairwise tree: 192->96->...->3
                    h = w // 2
                    nc.vector.tensor_tensor(
                        p3[:, 0:d, 0:h], p3[:, 0:d, 0:h], p3[:, 0:d, h:w],
                        mybir.AluOpType.add)
                    w = h
                nc.vector.tensor_reduce(
                    qt_t[:, base:base + d], p3[:, 0:d, 0:3],
                    axis=mybir.AxisListType.X, op=mybir.AluOpType.add)
            for g in range(d, G):
                jt = junk_p.tile([P, S], f16)
                nc.scalar.activation(
                    jt[:], pt[:, g * S:(g + 1) * S], Cpy,
                    accum_out=qt_t[:, base + g:base + g + 1])

        for it in range(3):
            qt1 = qt_p.tile([P, S], f32, tag="qt1")
            qt2 = qt_p.tile([P, 96], f32, tag="qt2")
            last = it == 2
            if not last:
                nsa = sig_p.tile([P, S], f16, tag="siga")
                nsb = sig_p.tile([P, S], f16, tag="sigb")
                sq1 = sq_p.tile([P, 256], f16, tag="sq1")
                sq2 = sq_p.tile([P, 128], f16, tag="sq2")
                tmp1 = sq_p.tile([P, 128], f16, tag="tmp1")
                tmp2 = sq_p.tile([P, 128], f16, tag="tmp2")

            for si, s in enumerate(SLAB_ORDER[0:2]):   # chunk2 slabs first
                sp = "g" if it == 0 else ("k" if si == 0 else None)
                do_slab(s, siga_t, sigb_t, qt1, qt2, split=sp)
            nc.vector.tensor_tensor(qt2[:], qt2[:], sconT2p_t[:], mybir.AluOpType.add)
            if not last:
                # chunk2 boundary work overlaps chunk1 compute below
                nc.scalar.activation(sq2[:, 0:96], qt2[:], Sig)
                nc.scalar.activation(sq2[:, 96:128], qt2[:, 0:32], Sig)  # filler
                nc.sync.dma_start_transpose(tmp2[:], sq2[:])
                nc.scalar.dma_start(nsa[0:96, 128:192], tmp2[0:96, 0:64])
                nc.scalar.dma_start(nsa[96:128, 128:192], tmp2[0:32, 64:128])
                nc.scalar.dma_start(nsb[0:64, 128:192], tmp2[32:96, 64:128])
                nc.scalar.dma_start(nsb[64:128, 128:192], tmp2[32:96, 64:128])
            else:
                o2 = out_p.tile([P, 96], f32, tag="o2")
                nc.scalar.activation(o2[:], qt2[:], Sig)
                nc.sync.dma_start(qt_d.ap()[128:192, 0:96], o2[0:64, :])
                nc.sync.dma_start(qt_d.ap()[128:192, 96:192], o2[64:128, :])

            for s in SLAB_ORDER[2:]:            # chunk1 slabs
                do_slab(s, siga_t, sigb_t, qt1, qt2)
            nc.vector.tensor_tensor(qt1[:], qt1[:], sconT1_t[:], mybir.AluOpType.add)
            if not last:
                nc.scalar.activation(sq1[:, 0:S], qt1[:], Sig)
                nc.scalar.activation(sq1[:, S:256], qt1[:, 0:64], Sig)  # filler
                nc.sync.dma_start_transpose(nsa[0:128, 0:128], sq1[:, 0:128])
                nc.sync.dma_start_transpose(tmp1[:], sq1[:, 128:256])
                nc.scalar.dma_start(nsb[0:64, 0:128], tmp1[0:64, :])
                nc.scalar.dma_start(nsb[64:128, 0:128], tmp1[0:64, :])
                siga_t, sigb_t = nsa, nsb
            else:
                o1 = out_p.tile([P, S], f32, tag="o1")
                nc.scalar.activation(o1[:], qt1[:], Sig)
                nc.sync.dma_start(qt_d.ap()[0:128, :], o1[:])
    nc.compile()
    return nc


def _get_program():
    if "nc" not in _CACHE:
        _CACHE["nc"] = _build_program()
    return _CACHE["nc"]


def _prep_core_inputs(s_con_b, sbm16_b):
    """Per-batch input dict. sbm16_b: masked s_bin, fp16, [i, j, k]."""
    A = sbm16_b
    c1 = np.ascontiguousarray(A[:, 0:128, :].transpose(1, 0, 2)).reshape(P, S * S)
    c2a = A[0:96, 128:192, :].transpose(1, 0, 2)     # [64, 96, 192]
    c2b = A[96:192, 128:192, :].transpose(1, 0, 2)   # [64, 96, 192]
    c2 = np.ascontiguousarray(np.concatenate([c2a, c2b], 0)).reshape(P, 96 * S)
    sig1 = (1.0 / (1.0 + np.exp(-s_con_b))).astype(np.float16)   # [a, k] natural
    siga = np.ascontiguousarray(sig1[0:128])
    sigb = np.ascontiguousarray(np.concatenate([sig1[128:192]] * 2, 0))
    sconT = np.ascontiguousarray(s_con_b.T)          # [j, i]
    sconT1 = sconT[0:128].copy()
    sconT2p = np.concatenate([sconT[128:192, 0:96], sconT[128:192, 96:192]], 0).copy()
    return {"c1": c1, "c2": c2, "siga": siga, "sigb": sigb,
            "sconT1": sconT1, "sconT2p": sconT2p}


def kernel(s_con, s_bin, mask):
    from concourse.bass_utils import run_bass_kernel_spmd

    s_con = np.asarray(s_con, dtype=np.float32)
    s_bin = np.asarray(s_bin, dtype=np.float32)
    mask = np.asarray(mask)

    idx = np.arange(S)
    ne = idx[:, None] != idx[None, :]                       # [a, k]
    m2 = ne[:, None, :] & ne[None, :, :]                    # [i, j, k]
    full_mask = mask[:, :, :, None] & m2[None]              # [B, i, j, k]
    sbm16 = (s_bin * full_mask).astype(np.float16)

    nc = _get_program()
    in_maps = [_prep_core_inputs(s_con[b], sbm16[b]) for b in range(B)]
    res = run_bass_kernel_spmd(nc, in_maps, list(range(B)))
    out = np.stack([res.results[b]["qt_out"].T for b in range(B)], 0)
    return np.ascontiguousarray(out.astype(np.float32))



# revision 2
# speedup vs baseline: 2.9796x; 2.9796x over previous
"""Trainium2 Bass kernel for MFVIConstituency mean-field iterations.

Per batch b (one NeuronCore each, 8 total):
    q = s_con;  repeat 3x:  q[i,j] = s_con[i,j] + sum_k sig(q)[j,k] * sb[i,j,k]
    out = sigmoid(q)
where sb = s_bin * mask2o, mask2o[i,j,k] = mask[i,j] & (i!=k) & (j!=k).

Scheme: k lives on SBUF partitions; the elementwise product p = sb * T
(T[k,j] = sig(q)[j,k]) is computed on DVE + Pool + (iters 2-3) ACT; the
k-reduction runs on the otherwise-idle PE as weights-stationary matmuls:
each call loads a 64..128-column block of p as weights and streams a
single ones column, producing one PSUM column of segment sums.

PSUM protocol per bank per iteration: one opener matmul (start=True) with
identity weights writes s_con^T into the used region — marking the 2KB
zero-region, seeding q with s_con, and W->W-ordering every column call
after it; all column calls then accumulate with start=False.
sigmoid reads PSUM directly; its fp16 output IS the next iteration's T
tile (the [j_p, i] psum layout makes T_head = sig(psum1) verbatim).

The head product is split by j into two regions:
  sbh  [128 k, (i, jj 0:128)]  with jj = [j 0:64, j 128:192]
  sbact [128 k, (i, j' 0:64)]  with j' = j - 64  (the "ACT region")
In iters 2-3 the ACT engine multiplies ACT_COLS of the j' columns using
per-partition-scalar activations (scale = Th[:, 64+c]), relieving
DVE/Pool.  Tails (k 128:192) stay i-parity packed:
  sbt [64h+k' (h = i parity), (ipair, j 0:192)].
psum1 [j 0:128, i] (A1 -> rows 0:64, A2 -> rows 64:128, tail-A all),
psum2 [64 = j-128, i].

The initial sb load is split across the three DMA-capable queues
(SP, ACT, Pool) so it takes ~20us instead of ~43us; iteration 1 gives
DVE a larger share of the product since Pool and ACT are DMA queues.
"""

import numpy as np

S = 192
B = 8
G = 48             # i-values per slab; 4 slabs
NSLAB = S // G
GD1, GT1 = 32, 14  # iter-1 DVE head/tail rows per slab
GD, GT = 30, 14    # iters 2-3 DVE head/tail rows per slab
ACT_COLS = 42      # iters 2-3: j' columns multiplied on ACT
JSPLIT = 110       # iters 2-3: DVE rows of the leftover j' region (of 192)

_CACHE = {}


def _build_program():
    import concourse.tile as tile
    from concourse import mybir, bacc
    from contextlib import ExitStack

    f16, f32 = mybir.dt.float16, mybir.dt.float32
    Sig = mybir.ActivationFunctionType.Sigmoid
    Cpy = mybir.ActivationFunctionType.Copy
    Mult = mybir.AluOpType.mult

    nc = bacc.Bacc("TRN2", target_bir_lowering=False, debug=False, num_devices=B)
    sbh_d = nc.dram_tensor("sbh", [128, S * 128], f16, kind="ExternalInput")
    sbact_d = nc.dram_tensor("sbact", [128, S * 64], f16, kind="ExternalInput")
    sbt_d = nc.dram_tensor("sbt", [128, (S // 2) * S], f16, kind="ExternalInput")
    Th0_d = nc.dram_tensor("Th0", [128, S], f16, kind="ExternalInput")
    Tt0_d = nc.dram_tensor("Tt0", [128, S], f16, kind="ExternalInput")
    ident_d = nc.dram_tensor("ident", [128, 128], f16, kind="ExternalInput")
    sconT1_d = nc.dram_tensor("sconT1", [128, S], f16, kind="ExternalInput")
    sconT2_d = nc.dram_tensor("sconT2", [64, S], f16, kind="ExternalInput")
    o1_d = nc.dram_tensor("o1", [128, S], f32, kind="ExternalOutput")
    o2_d = nc.dram_tensor("o2", [64, S], f32, kind="ExternalOutput")

    with tile.TileContext(nc) as tc, ExitStack() as ctx:
        sb_p = ctx.enter_context(tc.tile_pool(name="sb", bufs=1))
        small_p = ctx.enter_context(tc.tile_pool(name="small", bufs=1))
        T_p = ctx.enter_context(tc.tile_pool(name="T", bufs=2))
        p_p = ctx.enter_context(tc.tile_pool(name="p", bufs=1))
        pa_p = ctx.enter_context(tc.tile_pool(name="pa", bufs=1))
        ps_p = ctx.enter_context(tc.psum_pool(name="ps", bufs=2))
        o_p = ctx.enter_context(tc.tile_pool(name="o", bufs=1))

        ones = small_p.tile([128, 1], f16, tag="ones")
        nc.vector.memset(ones[:], 1.0)
        # preload the sigmoid activation table before anything else on ACT
        junk = small_p.tile([128, 1], f16, tag="junk")
        nc.scalar.activation(junk[:], ones[:], Sig)
        Th = T_p.tile([128, S], f16, tag="Th")
        nc.scalar.dma_start(Th[:], Th0_d.ap())
        Tt = T_p.tile([128, S], f16, tag="Tt")
        nc.scalar.dma_start(Tt[:], Tt0_d.ap())
        ident = small_p.tile([128, 128], f16, tag="ident")
        nc.gpsimd.dma_start(ident[:], ident_d.ap())
        sconT1 = small_p.tile([128, S], f16, tag="sconT1")
        nc.gpsimd.dma_start(sconT1[:], sconT1_d.ap())
        sconT2 = small_p.tile([64, S], f16, tag="sconT2")
        nc.gpsimd.dma_start(sconT2[:], sconT2_d.ap())

        # --- sb load, spread over SP + ACT + Pool queues ---
        sbh = sb_p.tile([128, S * 128], f16, tag="sbh")
        sbact = sb_p.tile([128, S * 64], f16, tag="sbact")
        sbt = sb_p.tile([128, (S // 2) * S], f16, tag="sbt")

        def load(eng, t, td, i0, i1, w):
            eng.dma_start(t[:, i0 * w:i1 * w], td.ap()[:, i0 * w:i1 * w])

        # SP: DVE pieces (sbh + sbact), half-slab granularity
        for s in range(NSLAB):
            i0 = s * G
            load(nc.sync, sbh, sbh_d, i0, i0 + GD1 // 2, 128)
            load(nc.sync, sbact, sbact_d, i0, i0 + GD1 // 2, 64)
            load(nc.sync, sbh, sbh_d, i0 + GD1 // 2, i0 + GD1, 128)
            load(nc.sync, sbact, sbact_d, i0 + GD1 // 2, i0 + GD1, 64)
        # ACT: slab-0 Pool piece, the tails, slab-1 Pool piece
        load(nc.scalar, sbh, sbh_d, GD1, G, 128)
        load(nc.scalar, sbact, sbact_d, GD1, G, 64)
        for s in range(NSLAB):
            c0, c1 = (s * G // 2) * S, ((s * G + G) // 2) * S
            nc.scalar.dma_start(sbt[:, c0:c1], sbt_d.ap()[:, c0:c1])
            if s == 0:
                load(nc.scalar, sbh, sbh_d, G + GD1, 2 * G, 128)
                load(nc.scalar, sbact, sbact_d, G + GD1, 2 * G, 64)
        # Pool: its own pieces for slabs 2 and 3
        for s in (2, 3):
            load(nc.gpsimd, sbh, sbh_d, s * G + GD1, (s + 1) * G, 128)
            load(nc.gpsimd, sbact, sbact_d, s * G + GD1, (s + 1) * G, 64)

        sbh3 = sbh[:].rearrange("p (g k) -> p g k", k=128)    # [128, i, jj]
        sba3 = sbact[:].rearrange("p (g k) -> p g k", k=64)   # [128, i, j']
        sbt3 = sbt[:].rearrange("p (g k) -> p g k", k=S)      # [128, ip, j]

        def bc(t, c0, c1, rows):
            return t[:, c0:c1].unsqueeze(1).broadcast_to([128, rows, c1 - c0])

        for it in range(3):
            last = it == 2
            gd, gt = (GD1, GT1) if it == 0 else (GD, GT)
            ps1 = ps_p.tile([128, 512], f32, tag="ps1")
            ps2 = ps_p.tile([64, 512], f32, tag="ps2")
            psum1, psum2 = ps1[:, 0:S], ps2[:, 0:S]
            # openers: seed q^T = s_con^T, mark zero-regions, order the bank
            nc.tensor.matmul(psum1, ident[:], sconT1[:],
                             start=True, stop=False, skip_group_check=True)
            nc.tensor.matmul(psum2, ident[0:64, 0:64], sconT2[:],
                             start=True, stop=False, skip_group_check=True)

            # ACT-region product pact[k, (i, j')]
            pact = pa_p.tile([128, S * 64], f16, tag="pact")
            pac3 = pact[:].rearrange("p (g k) -> p g k", k=64)
            if it > 0:
                for c in range(ACT_COLS):
                    nc.scalar.activation(pac3[:, :, c], sba3[:, :, c], Cpy,
                                         scale=ThS[:, c:c + 1])
                # leftover j' columns on DVE/Pool, split by i-rows
                nc.vector.tensor_tensor(
                    pac3[:, 0:JSPLIT, ACT_COLS:64],
                    sba3[:, 0:JSPLIT, ACT_COLS:64],
                    bc(Th, 64 + ACT_COLS, 128, JSPLIT), Mult)
                nc.gpsimd.tensor_tensor(
                    pac3[:, JSPLIT:S, ACT_COLS:64],
                    sba3[:, JSPLIT:S, ACT_COLS:64],
                    bc(Th, 64 + ACT_COLS, 128, S - JSPLIT), Mult)

            for s in range(NSLAB):
                i0 = s * G
                ph = p_p.tile([128, G * 128], f16, tag="ph", bufs=2)
                pt = p_p.tile([128, (G // 2) * S], f16, tag="pt", bufs=2)
                ph3 = ph[:].rearrange("p (g k) -> p g k", k=128)
                pt3 = pt[:].rearrange("p (g k) -> p g k", k=S)

                def head_mult(eng, r0, r1):
                    if r0 >= r1:
                        return
                    eng.tensor_tensor(ph3[:, r0:r1, 0:64],
                                      sbh3[:, i0 + r0:i0 + r1, 0:64],
                                      bc(Th, 0, 64, r1 - r0), Mult)
                    eng.tensor_tensor(ph3[:, r0:r1, 64:128],
                                      sbh3[:, i0 + r0:i0 + r1, 64:128],
                                      bc(Th, 128, 192, r1 - r0), Mult)
                    if it == 0:
                        eng.tensor_tensor(pac3[:, i0 + r0:i0 + r1, :],
                                          sba3[:, i0 + r0:i0 + r1, :],
                                          bc(Th, 64, 128, r1 - r0), Mult)

                if it == 0:
                    h = gd // 2   # match the split DMA pieces
                    head_mult(nc.vector, 0, h)
                    head_mult(nc.vector, h, gd)
                else:
                    head_mult(nc.vector, 0, gd)
                head_mult(nc.gpsimd, gd, G)
                # tail product: DVE rows 0:gt, Pool rows gt:G//2
                t0 = i0 // 2
                nc.vector.tensor_tensor(pt3[:, 0:gt, :],
                                        sbt3[:, t0:t0 + gt, :],
                                        Tt[:].unsqueeze(1).broadcast_to(
                                            [128, gt, S]), Mult)
                nc.gpsimd.tensor_tensor(pt3[:, gt:G // 2, :],
                                        sbt3[:, t0 + gt:t0 + G // 2, :],
                                        Tt[:].unsqueeze(1).broadcast_to(
                                            [128, G // 2 - gt, S]), Mult)

                for il in range(G):
                    i = i0 + il
                    nc.tensor.matmul(psum1[0:64, i:i + 1], ph3[:, il, 0:64],
                                     ones[:], start=False, stop=False,
                                     skip_group_check=True)
                    if it == 0:
                        # pact rows are produced slab-locally in iter 1
                        nc.tensor.matmul(psum1[64:128, i:i + 1], pac3[:, i, :],
                                         ones[:], start=False, stop=False,
                                         skip_group_check=True)
                    nc.tensor.matmul(psum2[:, i:i + 1], ph3[:, il, 64:128],
                                     ones[:], start=False, stop=False,
                                     skip_group_check=True)
                for ipl in range(G // 2):
                    ip = i0 // 2 + ipl
                    for h in range(2):
                        i = 2 * ip + h
                        hs = slice(64 * h, 64 * h + 64)
                        nc.tensor.matmul(psum1[:, i:i + 1], pt3[hs, ipl, 0:128],
                                         ones[hs, :], start=False, stop=False,
                                         skip_group_check=True)
                        nc.tensor.matmul(psum2[:, i:i + 1], pt3[hs, ipl, 128:192],
                                         ones[hs, :], start=False, stop=False,
                                         skip_group_check=True)

            if it > 0:
                # A2 calls last on the PE queue: they wait for the full
                # ACT-produced pact and must not block the per-slab calls
                for i in range(S):
                    nc.tensor.matmul(psum1[64:128, i:i + 1], pac3[:, i, :],
                                     ones[:], start=False, stop=False,
                                     skip_group_check=True)

            # sigmoid straight out of PSUM
            if not last:
                Th = T_p.tile([128, S], f16, tag="Th")
                Tt = T_p.tile([128, S], f16, tag="Tt")
                ThS = T_p.tile([128, ACT_COLS], f32, tag="ThS")
                nc.scalar.activation(Th[:], psum1, Sig)
                nc.scalar.activation(ThS[:], ps1[:, 64:64 + ACT_COLS], Sig)
                nc.scalar.activation(Tt[0:64, :], psum2, Sig)
                nc.scalar.activation(Tt[64:128, :], psum2, Sig)
            else:
                o1 = o_p.tile([128, S], f32, tag="o1")
                o2 = o_p.tile([64, S], f32, tag="o2")
                nc.scalar.activation(o1[:], psum1, Sig)
                nc.scalar.activation(o2[:], psum2, Sig)
                nc.sync.dma_start(o1_d.ap(), o1[:])
                nc.scalar.dma_start(o2_d.ap(), o2[:])
    nc.compile()
    return nc


def _get_program():
    if "nc" not in _CACHE:
        _CACHE["nc"] = _build_program()
    return _CACHE["nc"]


_IDENT = np.eye(128, dtype=np.float16)


def _prep_core_inputs(s_con_b, sbm16_b):
    """Per-batch input dict. sbm16_b: masked s_bin, fp16, [i, j, k]."""
    A = sbm16_b
    Ah = A[:, :, 0:128]                           # [i, j, k 0:128]
    sbh = np.ascontiguousarray(np.concatenate(
        [Ah[:, 0:64, :], Ah[:, 128:192, :]], axis=1
    ).transpose(2, 0, 1)).reshape(128, S * 128)
    sbact = np.ascontiguousarray(
        Ah[:, 64:128, :].transpose(2, 0, 1)).reshape(128, S * 64)
    tail = A[:, :, 128:192]                       # [i, j, 64]
    t_even = tail[0::2].transpose(2, 0, 1)        # [64, S/2, S]
    t_odd = tail[1::2].transpose(2, 0, 1)
    sbt = np.ascontiguousarray(
        np.concatenate([t_even, t_odd], 0)).reshape(128, (S // 2) * S)
    sig0T = (1.0 / (1.0 + np.exp(-s_con_b))).T.astype(np.float16)  # [k, j]
    Th0 = np.ascontiguousarray(sig0T[0:128])
    Tt0 = np.ascontiguousarray(np.concatenate([sig0T[128:192]] * 2, 0))
    sconT = np.ascontiguousarray(s_con_b.T).astype(np.float16)     # [j, i]
    return {"sbh": sbh, "sbact": sbact, "sbt": sbt, "Th0": Th0, "Tt0": Tt0,
            "ident": _IDENT,
            "sconT1": sconT[0:128].copy(), "sconT2": sconT[128:192].copy()}


def kernel(s_con, s_bin, mask):
    from concourse.bass_utils import run_bass_kernel_spmd

    s_con = np.asarray(s_con, dtype=np.float32)
    s_bin = np.asarray(s_bin, dtype=np.float32)
    mask = np.asarray(mask)

    idx = np.arange(S)
    ne = idx[:, None] != idx[None, :]                       # [a, k]
    m2 = ne[:, None, :] & ne[None, :, :]                    # [i, j, k]
    full_mask = mask[:, :, :, None] & m2[None]              # [B, i, j, k]
    sbm16 = (s_bin * full_mask).astype(np.float16)

    nc = _get_program()
    in_maps = [_prep_core_inputs(s_con[b], sbm16[b]) for b in range(B)]
    res = run_bass_kernel_spmd(nc, in_maps, list(range(B)))
    out = np.empty((B, S, S), np.float32)
    for b in range(B):
        out[b, :, 0:128] = res.results[b]["o1"].T
        out[b, :, 128:192] = res.results[b]["o2"].T
    return np.ascontiguousarray(out)


# revision 3
# speedup vs baseline: 3.0068x; 1.0091x over previous
"""Trainium2 Bass kernel for MFVIConstituency mean-field iterations.

Per batch b (one NeuronCore each, 8 total):
    q = s_con;  repeat 3x:  q[i,j] = s_con[i,j] + sum_k sig(q)[j,k] * sb[i,j,k]
    out = sigmoid(q)
where sb = s_bin * mask2o, mask2o[i,j,k] = mask[i,j] & (i!=k) & (j!=k).

Scheme: k lives on SBUF partitions; the elementwise product p = sb * T
(T[k,j] = sig(q)[j,k]) is computed on DVE + Pool + (iters 2-3) ACT; the
k-reduction runs on the otherwise-idle PE as weights-stationary matmuls:
each call loads a 64..128-column block of p as weights and streams a
single ones column, producing one PSUM column of segment sums.

PSUM protocol per bank per iteration: one opener matmul (start=True) with
identity weights writes s_con^T into the used region — marking the 2KB
zero-region, seeding q with s_con, and W->W-ordering every column call
after it; all column calls then accumulate with start=False.
sigmoid reads PSUM directly; its fp16 output IS the next iteration's T
tile (the [j_p, i] psum layout makes T_head = sig(psum1) verbatim).

The head product is split by j into two regions:
  sbh  [128 k, (i, jj 0:128)]  with jj = [j 0:64, j 128:192]
  sbact [128 k, (i, j' 0:64)]  with j' = j - 64  (the "ACT region")
In iters 2-3 the ACT engine multiplies ACT_COLS of the j' columns using
per-partition-scalar activations (scale = Th[:, 64+c]), relieving
DVE/Pool.  Tails (k 128:192) stay i-parity packed:
  sbt [64h+k' (h = i parity), (ipair, j 0:192)].
psum1 [j 0:128, i] (A1 -> rows 0:64, A2 -> rows 64:128, tail-A all),
psum2 [64 = j-128, i].

The initial sb load is split across the three DMA-capable queues
(SP, ACT, Pool) so it takes ~20us instead of ~43us; iteration 1 gives
DVE a larger share of the product since Pool and ACT are DMA queues.
"""

import numpy as np

S = 192
B = 8
G = 48             # i-values per slab; 4 slabs
NSLAB = S // G
GD1, GT1 = 31, 14  # iter-1 DVE head/tail rows per slab
GD, GT = 30, 14    # iters 2-3 DVE head/tail rows per slab
ACT_COLS = 42      # iters 2-3: j' columns multiplied on ACT
JSPLIT = 110       # iters 2-3: DVE rows of the leftover j' region (of 192)

_CACHE = {}


def _build_program():
    import concourse.tile as tile
    from concourse import mybir, bacc
    from contextlib import ExitStack

    f16, f32 = mybir.dt.float16, mybir.dt.float32
    Sig = mybir.ActivationFunctionType.Sigmoid
    Cpy = mybir.ActivationFunctionType.Copy
    Mult = mybir.AluOpType.mult

    nc = bacc.Bacc("TRN2", target_bir_lowering=False, debug=False, num_devices=B)
    sbh_d = nc.dram_tensor("sbh", [128, S * 128], f16, kind="ExternalInput")
    sbact_d = nc.dram_tensor("sbact", [128, S * 64], f16, kind="ExternalInput")
    sbt_d = nc.dram_tensor("sbt", [128, (S // 2) * S], f16, kind="ExternalInput")
    Th0_d = nc.dram_tensor("Th0", [128, S], f16, kind="ExternalInput")
    Tt0_d = nc.dram_tensor("Tt0", [128, S], f16, kind="ExternalInput")
    ident_d = nc.dram_tensor("ident", [128, 128], f16, kind="ExternalInput")
    sconT1_d = nc.dram_tensor("sconT1", [128, S], f16, kind="ExternalInput")
    sconT2_d = nc.dram_tensor("sconT2", [64, S], f16, kind="ExternalInput")
    o1_d = nc.dram_tensor("o1", [128, S], f32, kind="ExternalOutput")
    o2_d = nc.dram_tensor("o2", [64, S], f32, kind="ExternalOutput")

    with tile.TileContext(nc) as tc, ExitStack() as ctx:
        sb_p = ctx.enter_context(tc.tile_pool(name="sb", bufs=1))
        small_p = ctx.enter_context(tc.tile_pool(name="small", bufs=1))
        T_p = ctx.enter_context(tc.tile_pool(name="T", bufs=2))
        p_p = ctx.enter_context(tc.tile_pool(name="p", bufs=1))
        pa_p = ctx.enter_context(tc.tile_pool(name="pa", bufs=1))
        ps_p = ctx.enter_context(tc.psum_pool(name="ps", bufs=2))
        o_p = ctx.enter_context(tc.tile_pool(name="o", bufs=1))

        ones = small_p.tile([128, 1], f16, tag="ones")
        nc.vector.memset(ones[:], 1.0)
        Th = T_p.tile([128, S], f16, tag="Th")
        nc.scalar.dma_start(Th[:], Th0_d.ap())
        Tt = T_p.tile([128, S], f16, tag="Tt")
        nc.scalar.dma_start(Tt[:], Tt0_d.ap())

        # --- sb load, spread over SP + ACT + Pool queues ---
        sbh = sb_p.tile([128, S * 128], f16, tag="sbh")
        sbact = sb_p.tile([128, S * 64], f16, tag="sbact")
        sbt = sb_p.tile([128, (S // 2) * S], f16, tag="sbt")

        def load(eng, t, td, i0, i1, w):
            eng.dma_start(t[:, i0 * w:i1 * w], td.ap()[:, i0 * w:i1 * w])

        # SP: DVE pieces (sbh + sbact), half-slab granularity
        for s in range(NSLAB):
            i0 = s * G
            load(nc.sync, sbh, sbh_d, i0, i0 + GD1 // 2, 128)
            load(nc.sync, sbact, sbact_d, i0, i0 + GD1 // 2, 64)
            load(nc.sync, sbh, sbh_d, i0 + GD1 // 2, i0 + GD1, 128)
            load(nc.sync, sbact, sbact_d, i0 + GD1 // 2, i0 + GD1, 64)
        # ACT: slab-0 Pool piece, the tails, slab-1 Pool piece
        load(nc.scalar, sbh, sbh_d, GD1, G, 128)
        load(nc.scalar, sbact, sbact_d, GD1, G, 64)
        for s in range(NSLAB):
            c0, c1 = (s * G // 2) * S, ((s * G + G) // 2) * S
            nc.scalar.dma_start(sbt[:, c0:c1], sbt_d.ap()[:, c0:c1])
            if s == 0:
                load(nc.scalar, sbh, sbh_d, G + GD1, 2 * G, 128)
                load(nc.scalar, sbact, sbact_d, G + GD1, 2 * G, 64)
        # Pool: its own pieces for slabs 2 and 3, then the opener inputs
        # (the psum openers don't gate anything until the first drain)
        for s in (2, 3):
            load(nc.gpsimd, sbh, sbh_d, s * G + GD1, (s + 1) * G, 128)
            load(nc.gpsimd, sbact, sbact_d, s * G + GD1, (s + 1) * G, 64)
        ident = small_p.tile([128, 128], f16, tag="ident")
        nc.gpsimd.dma_start(ident[:], ident_d.ap())
        sconT1 = small_p.tile([128, S], f16, tag="sconT1")
        nc.gpsimd.dma_start(sconT1[:], sconT1_d.ap())
        sconT2 = small_p.tile([64, S], f16, tag="sconT2")
        nc.gpsimd.dma_start(sconT2[:], sconT2_d.ap())
        # preload the sigmoid activation table after ACT's DMA queue — it
        # only has to finish before the first boundary sigmoid
        junk = small_p.tile([128, 1], f16, tag="junk")
        nc.scalar.activation(junk[:], ones[:], Sig)

        sbh3 = sbh[:].rearrange("p (g k) -> p g k", k=128)    # [128, i, jj]
        sba3 = sbact[:].rearrange("p (g k) -> p g k", k=64)   # [128, i, j']
        sbt3 = sbt[:].rearrange("p (g k) -> p g k", k=S)      # [128, ip, j]

        def bc(t, c0, c1, rows):
            return t[:, c0:c1].unsqueeze(1).broadcast_to([128, rows, c1 - c0])

        for it in range(3):
            last = it == 2
            gd, gt = (GD1, GT1) if it == 0 else (GD, GT)
            ps1 = ps_p.tile([128, 512], f32, tag="ps1")
            ps2 = ps_p.tile([64, 512], f32, tag="ps2")
            psum1, psum2 = ps1[:, 0:S], ps2[:, 0:S]
            # openers: seed q^T = s_con^T, mark zero-regions, order the bank
            nc.tensor.matmul(psum1, ident[:], sconT1[:],
                             start=True, stop=False, skip_group_check=True)
            nc.tensor.matmul(psum2, ident[0:64, 0:64], sconT2[:],
                             start=True, stop=False, skip_group_check=True)

            # ACT-region product pact[k, (i, j')]
            pact = pa_p.tile([128, S * 64], f16, tag="pact")
            pac3 = pact[:].rearrange("p (g k) -> p g k", k=64)
            if it > 0:
                for c in range(ACT_COLS):
                    nc.scalar.activation(pac3[:, :, c], sba3[:, :, c], Cpy,
                                         scale=ThS[:, c:c + 1])
                # leftover j' columns on DVE/Pool, split by i-rows
                nc.vector.tensor_tensor(
                    pac3[:, 0:JSPLIT, ACT_COLS:64],
                    sba3[:, 0:JSPLIT, ACT_COLS:64],
                    bc(Th, 64 + ACT_COLS, 128, JSPLIT), Mult)
                nc.gpsimd.tensor_tensor(
                    pac3[:, JSPLIT:S, ACT_COLS:64],
                    sba3[:, JSPLIT:S, ACT_COLS:64],
                    bc(Th, 64 + ACT_COLS, 128, S - JSPLIT), Mult)

            for s in range(NSLAB):
                i0 = s * G
                ph = p_p.tile([128, G * 128], f16, tag="ph", bufs=2)
                pt = p_p.tile([128, (G // 2) * S], f16, tag="pt", bufs=2)
                ph3 = ph[:].rearrange("p (g k) -> p g k", k=128)
                pt3 = pt[:].rearrange("p (g k) -> p g k", k=S)

                def head_mult(eng, r0, r1):
                    if r0 >= r1:
                        return
                    eng.tensor_tensor(ph3[:, r0:r1, 0:64],
                                      sbh3[:, i0 + r0:i0 + r1, 0:64],
                                      bc(Th, 0, 64, r1 - r0), Mult)
                    eng.tensor_tensor(ph3[:, r0:r1, 64:128],
                                      sbh3[:, i0 + r0:i0 + r1, 64:128],
                                      bc(Th, 128, 192, r1 - r0), Mult)
                    if it == 0:
                        eng.tensor_tensor(pac3[:, i0 + r0:i0 + r1, :],
                                          sba3[:, i0 + r0:i0 + r1, :],
                                          bc(Th, 64, 128, r1 - r0), Mult)

                if it == 0:
                    h = gd // 2   # match the split DMA pieces
                    head_mult(nc.vector, 0, h)
                    head_mult(nc.vector, h, gd)
                else:
                    head_mult(nc.vector, 0, gd)
                head_mult(nc.gpsimd, gd, G)
                # tail product: DVE rows 0:gt, Pool rows gt:G//2
                t0 = i0 // 2
                nc.vector.tensor_tensor(pt3[:, 0:gt, :],
                                        sbt3[:, t0:t0 + gt, :],
                                        Tt[:].unsqueeze(1).broadcast_to(
                                            [128, gt, S]), Mult)
                nc.gpsimd.tensor_tensor(pt3[:, gt:G // 2, :],
                                        sbt3[:, t0 + gt:t0 + G // 2, :],
                                        Tt[:].unsqueeze(1).broadcast_to(
                                            [128, G // 2 - gt, S]), Mult)

                for il in range(G):
                    i = i0 + il
                    nc.tensor.matmul(psum1[0:64, i:i + 1], ph3[:, il, 0:64],
                                     ones[:], start=False, stop=False,
                                     skip_group_check=True)
                    if it == 0:
                        # pact rows are produced slab-locally in iter 1
                        nc.tensor.matmul(psum1[64:128, i:i + 1], pac3[:, i, :],
                                         ones[:], start=False, stop=False,
                                         skip_group_check=True)
                    nc.tensor.matmul(psum2[:, i:i + 1], ph3[:, il, 64:128],
                                     ones[:], start=False, stop=False,
                                     skip_group_check=True)
                for ipl in range(G // 2):
                    ip = i0 // 2 + ipl
                    for h in range(2):
                        i = 2 * ip + h
                        hs = slice(64 * h, 64 * h + 64)
                        nc.tensor.matmul(psum1[:, i:i + 1], pt3[hs, ipl, 0:128],
                                         ones[hs, :], start=False, stop=False,
                                         skip_group_check=True)
                        nc.tensor.matmul(psum2[:, i:i + 1], pt3[hs, ipl, 128:192],
                                         ones[hs, :], start=False, stop=False,
                                         skip_group_check=True)

            if it > 0:
                # A2 calls last on the PE queue: they wait for the full
                # ACT-produced pact and must not block the per-slab calls
                for i in range(S):
                    nc.tensor.matmul(psum1[64:128, i:i + 1], pac3[:, i, :],
                                     ones[:], start=False, stop=False,
                                     skip_group_check=True)

            # sigmoid straight out of PSUM
            if not last:
                Th = T_p.tile([128, S], f16, tag="Th")
                Tt = T_p.tile([128, S], f16, tag="Tt")
                ThS = T_p.tile([128, ACT_COLS], f32, tag="ThS")
                nc.scalar.activation(Th[:], psum1, Sig)
                nc.scalar.activation(ThS[:], ps1[:, 64:64 + ACT_COLS], Sig)
                nc.scalar.activation(Tt[0:64, :], psum2, Sig)
                nc.scalar.activation(Tt[64:128, :], psum2, Sig)
            else:
                o1 = o_p.tile([128, S], f32, tag="o1")
                o2 = o_p.tile([64, S], f32, tag="o2")
                nc.scalar.activation(o1[:], psum1, Sig)
                nc.scalar.activation(o2[:], psum2, Sig)
                nc.sync.dma_start(o1_d.ap(), o1[:])
                nc.scalar.dma_start(o2_d.ap(), o2[:])
    nc.compile()
    return nc


def _get_program():
    if "nc" not in _CACHE:
        _CACHE["nc"] = _build_program()
    return _CACHE["nc"]


_IDENT = np.eye(128, dtype=np.float16)


def _prep_core_inputs(s_con_b, sbm16_b):
    """Per-batch input dict. sbm16_b: masked s_bin, fp16, [i, j, k]."""
    A = sbm16_b
    Ah = A[:, :, 0:128]                           # [i, j, k 0:128]
    sbh = np.ascontiguousarray(np.concatenate(
        [Ah[:, 0:64, :], Ah[:, 128:192, :]], axis=1
    ).transpose(2, 0, 1)).reshape(128, S * 128)
    sbact = np.ascontiguousarray(
        Ah[:, 64:128, :].transpose(2, 0, 1)).reshape(128, S * 64)
    tail = A[:, :, 128:192]                       # [i, j, 64]
    t_even = tail[0::2].transpose(2, 0, 1)        # [64, S/2, S]
    t_odd = tail[1::2].transpose(2, 0, 1)
    sbt = np.ascontiguousarray(
        np.concatenate([t_even, t_odd], 0)).reshape(128, (S // 2) * S)
    sig0T = (1.0 / (1.0 + np.exp(-s_con_b))).T.astype(np.float16)  # [k, j]
    Th0 = np.ascontiguousarray(sig0T[0:128])
    Tt0 = np.ascontiguousarray(np.concatenate([sig0T[128:192]] * 2, 0))
    sconT = np.ascontiguousarray(s_con_b.T).astype(np.float16)     # [j, i]
    return {"sbh": sbh, "sbact": sbact, "sbt": sbt, "Th0": Th0, "Tt0": Tt0,
            "ident": _IDENT,
            "sconT1": sconT[0:128].copy(), "sconT2": sconT[128:192].copy()}


def kernel(s_con, s_bin, mask):
    from concourse.bass_utils import run_bass_kernel_spmd

    s_con = np.asarray(s_con, dtype=np.float32)
    s_bin = np.asarray(s_bin, dtype=np.float32)
    mask = np.asarray(mask)

    idx = np.arange(S)
    ne = idx[:, None] != idx[None, :]                       # [a, k]
    m2 = ne[:, None, :] & ne[None, :, :]                    # [i, j, k]
    full_mask = mask[:, :, :, None] & m2[None]              # [B, i, j, k]
    sbm16 = (s_bin * full_mask).astype(np.float16)

    nc = _get_program()
    in_maps = [_prep_core_inputs(s_con[b], sbm16[b]) for b in range(B)]
    res = run_bass_kernel_spmd(nc, in_maps, list(range(B)))
    out = np.empty((B, S, S), np.float32)
    for b in range(B):
        out[b, :, 0:128] = res.results[b]["o1"].T
        out[b, :, 128:192] = res.results[b]["o2"].T
    return np.ascontiguousarray(out)
